# revision 3
# baseline (speedup 1.0000x reference)
"""Trainium2 Bass kernel for a 4-layer transformer encoder.

Problem shapes (hardcoded): B=2, T=2048, D=1024, H=16, DH=64, DFF=4096, L=4.

Sharding: token-parallel over 8 cores (512 tokens each; batch element b on
cores 4b..4b+3). Per layer each core computes q/k/v for its own tokens, then
AllGathers K and V inside its 4-core group, runs full attention for its 512
queries, and does Wo/LN/FFN locally. Activations are kept TRANSPOSED in SBUF
(xT: [D, tok]) so every matmul chains without on-device transposes; all
reductions over D (LayerNorm) are done with ones-vector matmuls on the PE.

Matmuls run as float32r (full PE rate at N>=512, fp32 storage). LayerNorm
gamma/beta and all biases are algebraically folded into weights / per-partition
bias vectors on the host.
"""

import sys

sys.path.insert(0, "/opt/trn_rl_repo")

import numpy as np

import concourse.bacc as bacc
import concourse.mybir as mybir
import concourse.tile as tile
from concourse.bass_utils import run_bass_kernel_spmd

F32 = mybir.dt.float32
F32R = mybir.dt.float32r
AF = mybir.ActivationFunctionType
ALU = mybir.AluOpType

L, D, H, DH, DFF = 4, 1024, 16, 64, 4096
B, T = 2, 2048
EPS = 1e-5
NCORES = 8
GROUP = 4              # cores per batch element
TOK = (B * T) // NCORES  # 512 tokens per core
P = 128
KD = D // P            # 8 d-tiles
KF = DFF // P          # 32 dff-tiles
NKT = (T // P)         # 16 k-token tiles per batch element
NPAIR = H // 2         # 8 head pairs

# vecs column layout (per layer): per-d-tile columns of folded bias vectors
C_BQ, C_BK, C_G, C_BY, C_F1B, C_G1, C_BY1 = 0, 8, 16, 24, 32, 64, 72
NVEC = 80

_PROGRAM = None  # cached compiled program
LAST = None      # BassKernelResults of the most recent run (for profiling)


def r32(ap):
    return ap.bitcast(F32R)


def _build_program():
    nc = bacc.Bacc("TRN2", target_bir_lowering=False, debug=False,
                   num_devices=NCORES)

    # ---- kernel I/O ----
    xT = nc.dram_tensor("xT", [KD, P, TOK], F32, kind="ExternalInput").ap()
    mb = nc.dram_tensor("mb", [P, NKT], F32, kind="ExternalInput").ap()
    wq = nc.dram_tensor("wq", [L, KD, P, D], F32, kind="ExternalInput").ap()
    wk = nc.dram_tensor("wk", [L, KD, P, D], F32, kind="ExternalInput").ap()
    wv = nc.dram_tensor("wv", [L, KD, P, D], F32, kind="ExternalInput").ap()
    wo = nc.dram_tensor("wo", [L, KD, 2, 64, 1024], F32,
                        kind="ExternalInput").ap()
    w1 = nc.dram_tensor("w1", [L, KF, P, D], F32, kind="ExternalInput").ap()
    w2 = nc.dram_tensor("w2", [L, KD, 4, P, 1024], F32,
                        kind="ExternalInput").ap()
    vecs = nc.dram_tensor("vecs", [L, P, NVEC], F32, kind="ExternalInput").ap()
    vecf = nc.dram_tensor("vecf", [P, 32], F32, kind="ExternalInput").ap()
    onesd = nc.dram_tensor("onesd", [P, 8], F32, kind="ExternalInput").ap()
    outT = nc.dram_tensor("outT", [KD, P, TOK], F32, kind="ExternalOutput").ap()

    rg = [[0, 1, 2, 3], [4, 5, 6, 7]]

    with tile.TileContext(nc) as tc:
        with tc.tile_pool(name="sb", bufs=1) as sb, \
             tc.tile_pool(name="ps", bufs=1, space="PSUM") as psp, \
             tc.tile_pool(name="dr", bufs=1, space="DRAM") as dr:

            # ---- constants (f32r matmul operands must be f32r-produced) ----
            ones_col = sb.tile([P, 1], F32, tag="ones_col", bufs=1)
            nc.sync.dma_start(r32(ones_col[:]), r32(onesd[:, 0:1]))
            ones_row = sb.tile([1, P], F32, tag="ones_row", bufs=1)
            nc.sync.dma_start(r32(ones_row[:]),
                              r32(onesd[:, 0:1].rearrange("p o -> o p")))
            mb_t = sb.tile([P, NKT], F32, tag="mb", bufs=1)
            nc.sync.dma_start(mb_t[:], mb[:])

            # current activation (transposed, pre-affine): 8 tiles [128, TOK]
            xh = []
            for k in range(KD):
                t = sb.tile([P, TOK], F32, tag=f"xh{k}", bufs=1, name=f"xh_{k}")
                nc.sync.dma_start(r32(t[:]), r32(xT[k]))
                xh.append(t)

            vec_t = None

            def proj_T(l, w_ap, bias_col, tag, big):
                """out[o] [128, TOK] = (W' @ y)^T tiles, bias per-partition."""
                outs = []
                for o in range(KD):
                    wt = sb.tile([P, D], F32, tag="wproj", bufs=2,
                                 name=f"w_{tag}_{l}_{o}")
                    nc.sync.dma_start(r32(wt[:]), r32(w_ap[l, o]))
                    ps = psp.tile([P, TOK], F32, tag="ps1", bufs=2,
                                  name=f"ps_{tag}_{l}_{o}")
                    for k in range(KD):
                        nc.tensor.matmul(ps[:], r32(wt[:, k * P:(k + 1) * P]),
                                         r32(xh[k][:]),
                                         start=(k == 0), stop=(k == KD - 1))
                    if big:
                        ot = sb.tile([P, TOK], F32, tag="big", bufs=32,
                                     name=f"{tag}_{l}_{o}")
                    else:
                        ot = sb.tile([P, TOK], F32, tag="kvout", bufs=2,
                                     name=f"{tag}_{l}_{o}")
                    nc.scalar.activation(r32(ot[:]), ps[:], AF.Identity,
                                         bias=vec_t[:, bias_col + o:bias_col + o + 1])
                    outs.append(ot)
                return outs

            for l in range(L):
                vec_t = sb.tile([P, NVEC], F32, tag="vec", bufs=2,
                                name=f"vec_{l}")
                nc.sync.dma_start(vec_t[:], vecs[l])

                # ---- K projection (transposed) + bounce + AllGather ----
                k_src = dr.tile([KD, P, TOK], F32, name=f"k_src_{l}")
                kts = proj_T(l, wk, C_BK, "kt", big=False)
                for o in range(KD):
                    nc.gpsimd.dma_start(k_src[o], kts[o][:])
                k_all = dr.tile([GROUP, KD, P, TOK], F32, name=f"k_all_{l}")
                nc.gpsimd.collective_compute(
                    "AllGather", ALU.bypass, replica_groups=rg,
                    ins=[k_src.opt()], outs=[k_all.opt()])

                # ---- V projection (normal layout [tok, d]), 4 col-quarters --
                v_src = dr.tile([16, P, 256], F32, name=f"v_src_{l}")
                for n in range(4):
                    wvt = []
                    for k in range(KD):
                        wvk = sb.tile([P, 256], F32, tag="sh4", bufs=8,
                                      name=f"wv_{l}_{n}_{k}")
                        nc.sync.dma_start(r32(wvk[:]),
                                          r32(wv[l, k][:, n * 256:(n + 1) * 256]))
                        wvt.append(wvk)
                    for t in range(4):
                        ps = psp.tile([P, 256], F32, tag="ps1", bufs=2,
                                      name=f"ps_v_{l}_{n}_{t}")
                        for k in range(KD):
                            nc.tensor.matmul(
                                ps[:], r32(xh[k][:, t * P:(t + 1) * P]),
                                r32(wvt[k][:]),
                                start=(k == 0), stop=(k == KD - 1))
                        vt = sb.tile([P, 256], F32, tag="kvout", bufs=2,
                                     name=f"v_{l}_{t}_{n}")
                        nc.scalar.copy(r32(vt[:]), ps[:])
                        nc.gpsimd.dma_start(v_src[t * 4 + n], vt[:])
                v_all = dr.tile([GROUP, 16, P, 256], F32, name=f"v_all_{l}")
                nc.gpsimd.collective_compute(
                    "AllGather", ALU.bypass, replica_groups=rg,
                    ins=[v_src.opt()], outs=[v_all.opt()])

                # ---- Q projection (overlaps the collectives) ----
                qts = proj_T(l, wq, C_BQ, "qt", big=True)

                # ---- attention, one head pair at a time ----
                ctxh = []
                for p in range(NPAIR):
                    ctx = psp.tile([65, 2 * TOK], F32, tag="ps2", bufs=3,
                                   name=f"ctx_{l}_{p}")
                    for kt in range(NKT):
                        c, j = kt // 4, kt % 4
                        ktile = sb.tile([P, P], F32, tag="ktile", bufs=3,
                                        name=f"ktile_{l}_{p}_{kt}")
                        nc.sync.dma_start(r32(ktile[:]),
                                          r32(k_all[c, p][:, j * P:(j + 1) * P]))
                        lg = psp.tile([P, 2 * TOK], F32, tag="ps2", bufs=3,
                                      name=f"lg_{l}_{p}_{kt}")
                        nc.tensor.matmul(lg[:, 0:TOK], r32(ktile[0:64, :]),
                                         r32(qts[p][0:64, :]))
                        nc.tensor.matmul(lg[:, TOK:2 * TOK],
                                         r32(ktile[64:128, :]),
                                         r32(qts[p][64:128, :]))
                        probs = sb.tile([P, 2 * TOK], F32, tag="probs", bufs=2,
                                        name=f"probs_{l}_{p}_{kt}")
                        nc.scalar.activation(r32(probs[:]), lg[:], AF.Exp,
                                             scale=0.125,
                                             bias=mb_t[:, kt:kt + 1])
                        vext = sb.tile([P, 130], F32, tag="vext", bufs=2,
                                       name=f"vext_{l}_{p}_{kt}")
                        vv = vext[:, 0:130].rearrange("q (h c) -> q h c", h=2)
                        nc.sync.dma_start(
                            r32(vv[:, :, 64:65]),
                            r32(onesd[:, 0:2]
                                .rearrange("p (h o) -> p h o", h=2)))
                        nc.sync.dma_start(
                            r32(vv[:, :, 0:64]),
                            r32(v_all[c, j * 4 + p // 2]
                                [:, (p % 2) * P:(p % 2) * P + P]
                                .rearrange("t (h c) -> t h c", h=2)))
                        nc.tensor.matmul(ctx[:, 0:TOK], r32(vext[:, 0:65]),
                                         r32(probs[:, 0:TOK]),
                                         start=(kt == 0), stop=(kt == NKT - 1))
                        nc.tensor.matmul(ctx[:, TOK:2 * TOK],
                                         r32(vext[:, 65:130]),
                                         r32(probs[:, TOK:2 * TOK]),
                                         start=(kt == 0), stop=(kt == NKT - 1))
                    # normalize: divide ctx rows 0..63 by the ones-row sums.
                    # Sums sit on PSUM partition 64: reciprocal lane-aligned
                    # on partition 64, then SBUF->SBUF DMA down to partition 0.
                    s64 = sb.tile([65, 2 * TOK], F32, tag="rbsb", bufs=2,
                                  name=f"s64_{l}_{p}")
                    nc.vector.reciprocal(s64[64:65, :], ctx[64:65, :])
                    stile = sb.tile([1, 2 * TOK], F32, tag="stB", bufs=1,
                                    name=f"sums_{l}_{p}")
                    nc.sync.dma_start(r32(stile[:, 0:2 * TOK]), r32(s64[64:65, :]))
                    rb = psp.tile([64, 2 * TOK], F32, tag="ps2", bufs=3,
                                  name=f"rb_{l}_{p}")
                    nc.tensor.matmul(rb[:, 0:TOK], r32(ones_row[:, 0:64]),
                                     r32(stile[:, 0:TOK]))
                    nc.tensor.matmul(rb[:, TOK:2 * TOK], r32(ones_row[:, 0:64]),
                                     r32(stile[:, TOK:2 * TOK]))
                    rb_sb = sb.tile([64, 2 * TOK], F32, tag="rbsb", bufs=2,
                                    name=f"rbsb_{l}_{p}")
                    nc.vector.tensor_copy(rb_sb[:], rb[:])
                    ch = sb.tile([64, 2 * TOK], F32, tag="sh4", bufs=8,
                                 name=f"ctxh_{l}_{p}")
                    nc.vector.tensor_mul(r32(ch[:]), ctx[0:64, :], rb_sb[:])
                    ctxh.append(ch)

                # ---- Wo projection + residual -> r1 ----
                r1 = []
                for o in range(KD):
                    ps = psp.tile([P, TOK], F32, tag="ps1", bufs=2,
                                  name=f"ps_wo_{l}_{o}")
                    for hf in range(2):
                        wot = sb.tile([64, 1024], F32, tag="wwo2", bufs=2,
                                      name=f"wo_{l}_{o}_{hf}")
                        nc.sync.dma_start(r32(wot[:]), r32(wo[l, o, hf]))
                        for i in range(8):
                            idx = hf * 8 + i
                            pp, half = idx // 2, idx % 2
                            nc.tensor.matmul(
                                ps[:], r32(wot[:, i * P:(i + 1) * P]),
                                r32(ctxh[pp][:, half * TOK:(half + 1) * TOK]),
                                start=(idx == 0), stop=(idx == 15))
                    tsy = sb.tile([P, TOK], F32, tag="tsy", bufs=2,
                                  name=f"tsy_{l}_{o}")
                    nc.vector.tensor_scalar(
                        tsy[:], xh[o][:], vec_t[:, C_G + o:C_G + o + 1],
                        vec_t[:, C_BY + o:C_BY + o + 1], ALU.mult, ALU.add)
                    rt = sb.tile([P, TOK], F32, tag=f"r{o}", bufs=1,
                                 name=f"r1_{l}_{o}")
                    nc.vector.tensor_add(r32(rt[:]), ps[:], tsy[:])
                    r1.append(rt)

                def layernorm(src, phase, out_cb=None):
                    """src: 8 tiles [128, TOK] -> 8 new pre-affine tiles."""
                    stats = psp.tile([1, 2 * TOK], F32, tag="ps2", bufs=3,
                                     name=f"stats_{phase}")
                    for k in range(KD):
                        nc.tensor.matmul(stats[:, 0:TOK], r32(ones_col[:]),
                                         r32(src[k][:]),
                                         start=(k == 0), stop=(k == KD - 1))
                    for k in range(KD):
                        sq = sb.tile([P, 2 * TOK], F32, tag="probs", bufs=2,
                                     name=f"sq_{phase}_{k}")
                        nc.scalar.square(r32(sq[:, 0:TOK]), src[k][:])
                        nc.tensor.matmul(stats[:, TOK:2 * TOK],
                                         r32(ones_col[:]), r32(sq[:, 0:TOK]),
                                         start=(k == 0), stop=(k == KD - 1))
                    # stA: plain-f32 scratch; stB: f32r-only writers
                    # (the verifier tracks f32r-ness per memory location).
                    stA = sb.tile([1, 3 * TOK], F32, tag="st", bufs=1,
                                  name=f"stA_{phase}")
                    stB = sb.tile([1, 2 * TOK], F32, tag="stB", bufs=1,
                                  name=f"stB_{phase}")
                    mean = stA[:, 0:TOK]
                    wk1 = stA[:, TOK:2 * TOK]     # msq -> var -> ln(var)
                    t3 = stA[:, 2 * TOK:3 * TOK]
                    rs = stB[:, 0:TOK]
                    murs = stB[:, TOK:2 * TOK]
                    nc.vector.tensor_scalar_mul(mean, stats[:, 0:TOK], 1.0 / D)
                    nc.vector.tensor_scalar_mul(wk1, stats[:, TOK:2 * TOK],
                                                1.0 / D)
                    nc.vector.tensor_mul(t3, mean, mean)
                    nc.vector.tensor_sub(wk1, wk1, t3)           # var
                    nc.vector.tensor_scalar_add(wk1, wk1, EPS)
                    nc.scalar.activation(wk1, wk1, AF.Ln)
                    nc.scalar.activation(r32(rs), wk1, AF.Exp, scale=-0.5)
                    nc.vector.tensor_mul(r32(murs), mean, rs)    # mu*rs
                    bc = psp.tile([P, 2 * TOK], F32, tag="ps2", bufs=3,
                                  name=f"bc_{phase}")
                    nc.tensor.matmul(bc[:, 0:TOK], r32(ones_row[:]), r32(rs))
                    nc.tensor.matmul(bc[:, TOK:2 * TOK], r32(ones_row[:]),
                                     r32(murs))
                    outs = []
                    for k in range(KD):
                        tb = sb.tile([P, TOK], F32, tag="tsy", bufs=2,
                                     name=f"lntmp_{phase}_{k}")
                        nc.vector.tensor_mul(tb[:], src[k][:], bc[:, 0:TOK])
                        if out_cb is None:
                            nt = sb.tile([P, TOK], F32, tag=f"xh{k}", bufs=1,
                                         name=f"xh_{phase}_{k}")
                            nc.vector.tensor_sub(r32(nt[:]), tb[:],
                                                 bc[:, TOK:2 * TOK])
                            outs.append(nt)
                        else:
                            out_cb(k, tb, bc)
                    return outs

                xh = layernorm(r1, f"ln1_{l}")

                # ---- FFN ----
                h_sb = []
                for f in range(KF):
                    wt = sb.tile([P, D], F32, tag="wfc1", bufs=2,
                                 name=f"w1_{l}_{f}")
                    nc.sync.dma_start(r32(wt[:]), r32(w1[l, f]))
                    ps = psp.tile([P, TOK], F32, tag="ps1", bufs=2,
                                  name=f"ps_f1_{l}_{f}")
                    for k in range(KD):
                        nc.tensor.matmul(ps[:], r32(wt[:, k * P:(k + 1) * P]),
                                         r32(xh[k][:]),
                                         start=(k == 0), stop=(k == KD - 1))
                    ht = sb.tile([P, TOK], F32, tag="big", bufs=32,
                                 name=f"h_{l}_{f}")
                    nc.scalar.activation(
                        r32(ht[:]), ps[:], AF.Gelu,
                        bias=vec_t[:, C_F1B + f:C_F1B + f + 1])
                    h_sb.append(ht)
                r2 = []
                for o in range(KD):
                    ps = psp.tile([P, TOK], F32, tag="ps1", bufs=2,
                                  name=f"ps_f2_{l}_{o}")
                    for qt in range(4):
                        wt = sb.tile([P, 1024], F32, tag="wfc2", bufs=2,
                                     name=f"w2_{l}_{o}_{qt}")
                        nc.sync.dma_start(r32(wt[:]), r32(w2[l, o, qt]))
                        for k in range(8):
                            kk = qt * 8 + k
                            nc.tensor.matmul(ps[:],
                                             r32(wt[:, k * P:(k + 1) * P]),
                                             r32(h_sb[kk][:]),
                                             start=(kk == 0), stop=(kk == 31))
                    tsy = sb.tile([P, TOK], F32, tag="tsy", bufs=2,
                                  name=f"tsy1_{l}_{o}")
                    nc.vector.tensor_scalar(
                        tsy[:], xh[o][:], vec_t[:, C_G1 + o:C_G1 + o + 1],
                        vec_t[:, C_BY1 + o:C_BY1 + o + 1], ALU.mult, ALU.add)
                    rt = sb.tile([P, TOK], F32, tag=f"r{o}", bufs=1,
                                 name=f"r2_{l}_{o}")
                    nc.vector.tensor_add(r32(rt[:]), ps[:], tsy[:])
                    r2.append(rt)

                xh = layernorm(r2, f"ln2_{l}")

            # ---- final: y3 = Gf*xh + BYf ; LNf ; apply lnf_g/b ; store ----
            vecf_t = sb.tile([P, 32], F32, tag="vec", bufs=2)
            nc.sync.dma_start(vecf_t[:], vecf[:])
            y3 = []
            for k in range(KD):
                yt = sb.tile([P, TOK], F32, tag=f"r{k}", bufs=1,
                             name=f"y3_{k}")
                nc.vector.tensor_scalar(
                    r32(yt[:]), xh[k][:], vecf_t[:, k:k + 1],
                    vecf_t[:, 8 + k:8 + k + 1], ALU.mult, ALU.add)
                y3.append(yt)

            def final_out(k, tb, bc):
                ot = sb.tile([P, TOK], F32, tag="big", bufs=32,
                             name=f"out_{k}")
                nc.vector.tensor_sub(ot[:], tb[:], bc[:, TOK:2 * TOK])
                ot2 = sb.tile([P, TOK], F32, tag="big", bufs=32,
                              name=f"out2_{k}")
                nc.vector.tensor_scalar(
                    ot2[:], ot[:], vecf_t[:, 16 + k:16 + k + 1],
                    vecf_t[:, 24 + k:24 + k + 1], ALU.mult, ALU.add)
                nc.sync.dma_start(outT[k], ot2[:])

            # reuse the last layer's layernorm helper via a tiny shim:
            # rebuild inline (identical math) for the final LN over y3.
            stats = psp.tile([1, 2 * TOK], F32, tag="ps2", bufs=3,
                             name="stats_f")
            for k in range(KD):
                nc.tensor.matmul(stats[:, 0:TOK], r32(ones_col[:]),
                                 r32(y3[k][:]),
                                 start=(k == 0), stop=(k == KD - 1))
            for k in range(KD):
                sq = sb.tile([P, 2 * TOK], F32, tag="probs", bufs=2,
                             name=f"sq_f_{k}")
                nc.scalar.square(r32(sq[:, 0:TOK]), y3[k][:])
                nc.tensor.matmul(stats[:, TOK:2 * TOK], r32(ones_col[:]),
                                 r32(sq[:, 0:TOK]),
                                 start=(k == 0), stop=(k == KD - 1))
            stA = sb.tile([1, 3 * TOK], F32, tag="st", bufs=1, name="stA_f")
            stB = sb.tile([1, 2 * TOK], F32, tag="stB", bufs=1, name="stB_f")
            mean = stA[:, 0:TOK]
            wk1 = stA[:, TOK:2 * TOK]
            t3 = stA[:, 2 * TOK:3 * TOK]
            rs = stB[:, 0:TOK]
            murs = stB[:, TOK:2 * TOK]
            nc.vector.tensor_scalar_mul(mean, stats[:, 0:TOK], 1.0 / D)
            nc.vector.tensor_scalar_mul(wk1, stats[:, TOK:2 * TOK], 1.0 / D)
            nc.vector.tensor_mul(t3, mean, mean)
            nc.vector.tensor_sub(wk1, wk1, t3)
            nc.vector.tensor_scalar_add(wk1, wk1, EPS)
            nc.scalar.activation(wk1, wk1, AF.Ln)
            nc.scalar.activation(r32(rs), wk1, AF.Exp, scale=-0.5)
            nc.vector.tensor_mul(r32(murs), mean, rs)
            bc = psp.tile([P, 2 * TOK], F32, tag="ps2", bufs=3, name="bc_f")
            nc.tensor.matmul(bc[:, 0:TOK], r32(ones_row[:]), r32(rs))
            nc.tensor.matmul(bc[:, TOK:2 * TOK], r32(ones_row[:]), r32(murs))
            for k in range(KD):
                tb = sb.tile([P, TOK], F32, tag="tsy", bufs=2,
                             name=f"ftmp_{k}")
                nc.vector.tensor_mul(tb[:], y3[k][:], bc[:, 0:TOK])
                final_out(k, tb, bc)

    nc.compile()
    return nc


def _get_program():
    global _PROGRAM
    if _PROGRAM is None:
        _PROGRAM = _build_program()
    return _PROGRAM


def _prep_host(inputs):
    """Fold affines/biases into weights; build DMA-friendly layouts."""
    f = lambda a: np.asarray(a, dtype=np.float64)
    Wq, bq = f(inputs["Wq"]), f(inputs["bq"])
    Wk, bk = f(inputs["Wk"]), f(inputs["bk"])
    Wv, bv = f(inputs["Wv"]), f(inputs["bv"])
    Wo, bo = f(inputs["Wo"]), f(inputs["bo"])
    f1w, f1b = f(inputs["fc1_w"]), f(inputs["fc1_b"])
    f2w, f2b = f(inputs["fc2_w"]), f(inputs["fc2_b"])
    ln1_g, ln1_b = f(inputs["ln1_g"]), f(inputs["ln1_b"])
    ln2_g, ln2_b = f(inputs["ln2_g"]), f(inputs["ln2_b"])
    lnf_g, lnf_b = f(inputs["lnf_g"]), f(inputs["lnf_b"])

    def proj_tiles(WT):  # WT [D_in, M_out] -> [M_out/P][P, D_in] lhsT blocks
        # block[o, p, k*P + c] = WT[k*P + p, o*P + c]
        kd = WT.shape[0] // P
        n_o = WT.shape[1] // P
        a = WT.reshape(kd, P, n_o, P).transpose(2, 1, 0, 3)  # [o, p, k, c]
        return np.ascontiguousarray(
            a.reshape(n_o, P, kd * P), dtype=np.float32)

    wq_h = np.empty((L, KD, P, D), np.float32)
    wk_h = np.empty((L, KD, P, D), np.float32)
    wv_h = np.empty((L, KD, P, D), np.float32)
    wo_h = np.empty((L, KD, 2, 64, 1024), np.float32)
    w1_h = np.empty((L, KF, P, D), np.float32)
    w2_h = np.empty((L, KD, 4, P, 1024), np.float32)
    vecs_h = np.empty((L, P, NVEC), np.float32)

    def cols(v):  # [dim] -> [P, dim/P] per-d-tile columns
        return np.ascontiguousarray(v.reshape(-1, P).T.astype(np.float32))

    g_in = np.ones(D)
    b_in = np.zeros(D)
    for l in range(L):
        bvp = bv[l] + Wv[l] @ b_in
        b_att = bo[l] + Wo[l] @ bvp
        bqp = bq[l] + Wq[l] @ b_in
        bkp = bk[l] + Wk[l] @ b_in
        f1bp = f1b[l] + f1w[l] @ ln1_b[l]
        wq_h[l] = proj_tiles((Wq[l] * g_in[None, :]).T)
        wk_h[l] = proj_tiles((Wk[l] * g_in[None, :]).T)
        # wv used as rhs tiles: wv_h[l, k] = WvT'[kP:(k+1)P, :]
        wv_h[l] = np.ascontiguousarray(
            (Wv[l] * g_in[None, :]).T.astype(np.float32)).reshape(KD, P, D)
        # wo as K=64 lhsT halves: wo_h[l, o, hf][p64, i*128 + c] =
        #   WoT[(hf*8 + i)*64 + p64, o*128 + c]
        WoT = Wo[l].T  # [1024 ctx-dims, 1024 out]
        a = WoT.reshape(2, 8, 64, KD, P).transpose(3, 0, 2, 1, 4)
        wo_h[l] = np.ascontiguousarray(
            a.reshape(KD, 2, 64, 1024), np.float32)
        w1_h[l] = proj_tiles((f1w[l] * ln1_g[l][None, :]).T)
        # fc2 lhsT blocks, split into 4 k-quarters per out-tile:
        # w2_h[l, o, qt][p, k*P + c] = fc2T[(qt*8 + k)*P + p, o*P + c]
        t2 = proj_tiles(f2w[l].T)  # [KD out][P, DFF]
        w2_h[l] = t2.reshape(KD, P, 4, 1024).transpose(0, 2, 1, 3)
        v = vecs_h[l]
        v[:, C_BQ:C_BQ + 8] = cols(bqp)
        v[:, C_BK:C_BK + 8] = cols(bkp)
        v[:, C_G:C_G + 8] = cols(g_in)
        v[:, C_BY:C_BY + 8] = cols(b_in + b_att)
        v[:, C_F1B:C_F1B + 32] = cols(f1bp)
        v[:, C_G1:C_G1 + 8] = cols(ln1_g[l])
        v[:, C_BY1:C_BY1 + 8] = cols(ln1_b[l] + f2b[l])
        g_in, b_in = ln2_g[l], ln2_b[l]

    vecf_h = np.empty((P, 32), np.float32)
    vecf_h[:, 0:8] = cols(g_in)
    vecf_h[:, 8:16] = cols(b_in)
    vecf_h[:, 16:24] = cols(lnf_g)
    vecf_h[:, 24:32] = cols(lnf_b)

    return dict(wq=wq_h, wk=wk_h, wv=wv_h, wo=wo_h, w1=w1_h, w2=w2_h,
                vecs=vecs_h, vecf=vecf_h)


def kernel(**inputs):
    nc = _get_program()
    shared = _prep_host(inputs)
    x = np.asarray(inputs["x"], dtype=np.float32)
    mask = np.asarray(inputs["mask"])

    in_maps = []
    for c in range(NCORES):
        b, s = c // GROUP, c % GROUP
        xTc = np.ascontiguousarray(
            x[b, s * TOK:(s + 1) * TOK, :].T).reshape(KD, P, TOK)
        mbv = (mask[b].astype(np.float64) - 1.0) * 30.0
        mb_c = np.ascontiguousarray(mbv.reshape(NKT, P).T.astype(np.float32))
        m = dict(shared)
        m["xT"] = xTc
        m["mb"] = mb_c
        m["onesd"] = np.ones((P, 8), np.float32)
        in_maps.append(m)

    global LAST
    res = run_bass_kernel_spmd(nc, in_maps, list(range(NCORES)))
    LAST = res
    out = np.empty((B, T, D), np.float32)
    for c in range(NCORES):
        b, s = c // GROUP, c % GROUP
        oT = res.results[c]["outT"].reshape(D, TOK)
        out[b, s * TOK:(s + 1) * TOK, :] = oT.T
    return out



# revision 13
# speedup vs baseline: 1.7202x; 1.7202x over previous
"""Trainium2 Bass kernel for a 4-layer transformer encoder.

Problem shapes (hardcoded): B=2, T=2048, D=1024, H=16, DH=64, DFF=4096, L=4.

Sharding: token-parallel over 8 cores (512 tokens each; batch element b on
cores 4b..4b+3). Per layer each core computes q/k/v for its own tokens, then
AllGathers K and V inside its 4-core group, runs full attention for its 512
queries, and does Wo/LN/FFN locally. Activations are kept TRANSPOSED in SBUF
(xT: [D, tok]); reductions over D (LayerNorm) use ones-vector matmuls.

v2: weights + FFN/residual activations in bf16 (half DMA, FWL weight loads);
attention operands (q/k/v/probs) in fp8e4m3 (half collective payload, same
PE rate); K/V SBUF-resident via two bulk AllGathers unpacked with 4 DMAs
each (no DMA in the attention inner loop); Wo contraction 128 via stacked
head pairs; LayerNorm scalar chain kept in f32/f32r.
"""

import sys

sys.path.insert(0, "/opt/trn_rl_repo")

import ml_dtypes
import numpy as np

import concourse.bacc as bacc
import concourse.mybir as mybir
import concourse.tile as tile
from concourse.bass_utils import run_bass_kernel_spmd

F32 = mybir.dt.float32
F32R = mybir.dt.float32r
BF16 = mybir.dt.bfloat16
FP8 = mybir.dt.float8e4   # q/k operands
FP8V = mybir.dt.float8e5  # v/probs (e5m2: wide range, no exp overflow)
AF = mybir.ActivationFunctionType
ALU = mybir.AluOpType

L, D, H, DH, DFF = 4, 1024, 16, 64, 4096
B, T = 2, 2048
EPS = 1e-5
NCORES = 8
GROUP = 4              # cores per batch element
TOK = (B * T) // NCORES  # 512 tokens per core
P = 128
KD = D // P            # 8 d-tiles
KF = DFF // P          # 32 dff-tiles
NKT = (T // P)         # 16 k-token tiles per batch element
NPAIR = H // 2         # 8 head pairs
VW = H * 65            # V_sb columns per key tile (65-strided heads)

# vecs column layout (per layer): per-d-tile columns of folded bias vectors
C_BQ, C_BK, C_G, C_BY, C_F1B, C_G1, C_BY1 = 0, 8, 16, 24, 32, 64, 72
NVEC = 80

_PROGRAM = None  # cached compiled program
LAST = None      # BassKernelResults of the most recent run (for profiling)


def r32(ap):
    return ap.bitcast(F32R)


def _build_program():
    nc = bacc.Bacc("TRN2", target_bir_lowering=False, debug=False,
                   num_devices=NCORES)

    # ---- kernel I/O ----
    xT = nc.dram_tensor("xT", [KD, P, TOK], F32, kind="ExternalInput").ap()
    mb = nc.dram_tensor("mb", [P, NKT], F32, kind="ExternalInput").ap()
    wq = nc.dram_tensor("wq", [L, KD, P, D], BF16, kind="ExternalInput").ap()
    wk = nc.dram_tensor("wk", [L, KD, P, D], BF16, kind="ExternalInput").ap()
    wv = nc.dram_tensor("wv", [L, KD, P, D], BF16, kind="ExternalInput").ap()
    wo = nc.dram_tensor("wo", [L, KD, P, D], BF16, kind="ExternalInput").ap()
    w1 = nc.dram_tensor("w1", [L, KF, P, D], BF16, kind="ExternalInput").ap()
    w2 = nc.dram_tensor("w2", [L, KD, 4, P, 1024], BF16,
                        kind="ExternalInput").ap()
    vecs = nc.dram_tensor("vecs", [L, P, NVEC], F32, kind="ExternalInput").ap()
    vecf = nc.dram_tensor("vecf", [P, 32], F32, kind="ExternalInput").ap()
    onesd = nc.dram_tensor("onesd", [P, 8], F32, kind="ExternalInput").ap()
    outT = nc.dram_tensor("outT", [KD, P, TOK], F32, kind="ExternalOutput").ap()

    rg = [[0, 1, 2, 3], [4, 5, 6, 7]]

    with tile.TileContext(nc) as tc:
        with tc.tile_pool(name="sb", bufs=1) as sb, \
             tc.tile_pool(name="ps", bufs=1, space="PSUM") as psp, \
             tc.tile_pool(name="dr", bufs=1, space="DRAM") as dr:

            # ---- constants ----
            ones_col = sb.tile([P, 1], F32, tag="ones_col", bufs=1)
            nc.sync.dma_start(r32(ones_col[:]), r32(onesd[:, 0:1]))
            ones_cb = sb.tile([P, 1], BF16, tag="ones_cb", bufs=1)
            nc.vector.memset(ones_cb[:], 1.0)
            ones_row = sb.tile([1, P], F32, tag="ones_row", bufs=1)
            nc.sync.dma_start(r32(ones_row[:]),
                              r32(onesd[:, 0:1].rearrange("p o -> o p")))
            mb_t = sb.tile([P, NKT], F32, tag="mb", bufs=1)
            nc.sync.dma_start(mb_t[:], mb[:])

            # current activation (transposed, pre-affine): f32 + bf16 copy
            xh = []
            xb = []
            for k in range(KD):
                t = sb.tile([P, TOK], F32, tag=f"x{k}", bufs=1, name=f"xh_{k}")
                nc.sync.dma_start(r32(t[:]), r32(xT[k]))
                xh.append(t)
                tb = sb.tile([P, TOK], BF16, tag=f"xb{k}", bufs=1,
                             name=f"xb_{k}")
                nc.vector.tensor_copy(tb[:], t[:])
                xb.append(tb)

            vec_t = None

            def proj8(l, w_ap, tag, out_cb):
                """8 out-tiles [128, TOK] = W'@x (bf16 weights); cb(o, ps)."""
                for o in range(KD):
                    wt = sb.tile([P, D], BF16, tag="wproj", bufs=2,
                                 name=f"w_{tag}_{l}_{o}")
                    nc.sync.dma_start(wt[:], w_ap[l, o])
                    ps = psp.tile([P, TOK], F32, tag="ps1", bufs=2,
                                  name=f"ps_{tag}_{l}_{o}")
                    for k in range(KD):
                        nc.tensor.matmul(ps[:], wt[:, k * P:(k + 1) * P],
                                         xb[k][:],
                                         start=(k == 0), stop=(k == KD - 1))
                    out_cb(o, ps)

            for l in range(L):
                vec_t = sb.tile([P, NVEC], F32, tag="vec", bufs=2,
                                name=f"vec_{l}")
                nc.sync.dma_start(vec_t[:], vecs[l])

                # ---- K projection -> fp8 stage -> AllGather ----
                k_stage = sb.tile([P, NPAIR * TOK], FP8, tag="kstage", bufs=1,
                                  name=f"kstage_{l}")

                def k_out(o, ps):
                    nc.scalar.activation(
                        k_stage[:, o * TOK:(o + 1) * TOK], ps[:], AF.Identity,
                        bias=vec_t[:, C_BK + o:C_BK + o + 1])
                proj8(l, wk, "kt", k_out)
                k_src = dr.tile([P, NPAIR * TOK], FP8, name=f"k_src_{l}")
                nc.gpsimd.dma_start(k_src[:], k_stage[:])
                k_all = dr.tile([GROUP, P, NPAIR * TOK], FP8,
                                name=f"k_all_{l}")
                nc.gpsimd.collective_compute(
                    "AllGather", ALU.bypass, replica_groups=rg,
                    ins=[k_src.opt()], outs=[k_all.opt()])

                # ---- V projection -> fp8 65-stride stage -> AllGather ----
                # v_stage: [128 tok, 4 tt, 16 heads, 65] (col 64 = ones)
                v_stage = sb.tile([P, 4 * VW], FP8V, tag="vstage", bufs=1,
                                  name=f"vstage_{l}")
                vsv = v_stage[:].rearrange("p (t h c) -> p t h c", t=4, h=H)
                nc.vector.memset(vsv[:, :, :, 64:65], 1.0)
                for n in range(2):
                    wvk = []
                    for k in range(KD):
                        wt = sb.tile([P, 512], BF16, tag="wv512", bufs=8,
                                     name=f"wvk_{l}_{n}_{k}")
                        nc.sync.dma_start(wt[:],
                                          wv[l, k][:, n * 512:(n + 1) * 512])
                        wvk.append(wt)
                    for t in range(4):
                        ps = psp.tile([P, 512], F32, tag="ps1", bufs=2,
                                      name=f"ps_v_{l}_{n}_{t}")
                        for k in range(KD):
                            nc.tensor.matmul(
                                ps[:], xb[k][:, t * P:(t + 1) * P],
                                wvk[k][:],
                                start=(k == 0), stop=(k == KD - 1))
                        nc.vector.tensor_copy(
                            vsv[:, t, n * 8:(n + 1) * 8, 0:64],
                            ps[:].rearrange("p (h c) -> p h c", h=8))
                v_src = dr.tile([P, 4 * VW], FP8V, name=f"v_src_{l}")
                nc.gpsimd.dma_start(v_src[:], v_stage[:])
                v_all = dr.tile([GROUP, P, 4 * VW], FP8V, name=f"v_all_{l}")
                nc.gpsimd.collective_compute(
                    "AllGather", ALU.bypass, replica_groups=rg,
                    ins=[v_src.opt()], outs=[v_all.opt()])

                # ---- Q projection -> fp8 ----
                qb = [sb.tile([P, TOK], FP8, tag="qb", bufs=8,
                              name=f"qb_{l}_{o}") for o in range(KD)]

                def q_out(o, ps):
                    nc.scalar.activation(
                        qb[o][:], ps[:], AF.Identity,
                        bias=vec_t[:, C_BQ + o:C_BQ + o + 1])
                proj8(l, wq, "qt", q_out)

                # ---- unpack gathers into resident K_sb / V_sb ----
                K_sb = sb.tile([P, NPAIR * 2048], FP8, tag="K_sb", bufs=1,
                               name=f"K_sb_{l}")
                kdst = K_sb[:].rearrange("d (p t) -> d p t", p=NPAIR)
                for c in range(GROUP):
                    nc.sync.dma_start(
                        kdst[:, :, c * TOK:(c + 1) * TOK],
                        k_all[c].rearrange("d (p t) -> d p t", p=NPAIR))
                V_sb = sb.tile([P, NKT * VW], FP8V, tag="V_sb", bufs=1,
                               name=f"V_sb_{l}")
                for c in range(GROUP):
                    nc.sync.dma_start(
                        V_sb[:, c * 4 * VW:(c + 1) * 4 * VW], v_all[c])

                # ---- attention ----
                stk = []
                for p in range(NPAIR):
                    # logits + exp for both heads of pair p, all 16 kt
                    probs = sb.tile([P, NKT * 2 * TOK], FP8V, tag="probs",
                                    bufs=2, name=f"probs_{l}_{p}")
                    for kt in range(NKT):
                        lg = psp.tile([P, 2 * TOK], F32, tag="lg", bufs=2,
                                      name=f"lg_{l}_{p}_{kt}")
                        for a in range(2):
                            nc.tensor.matmul(
                                lg[:, a * TOK:(a + 1) * TOK],
                                K_sb[a * 64:a * 64 + 64,
                                     p * 2048 + kt * P:
                                     p * 2048 + (kt + 1) * P],
                                qb[p][a * 64:a * 64 + 64, :])
                        nc.scalar.activation(
                            probs[:, kt * 2 * TOK:(kt + 1) * 2 * TOK],
                            lg[:], AF.Exp, scale=0.125,
                            bias=mb_t[:, kt:kt + 1])
                    # ctx for both heads; stack into [128, TOK] bf16
                    st = sb.tile([P, TOK], BF16, tag="stk", bufs=8,
                                 name=f"stk_{l}_{p}")
                    for a in range(2):
                        h = 2 * p + a
                        ctx = psp.tile([65, TOK], F32, tag="ctx", bufs=2,
                                       name=f"ctx_{l}_{h}")
                        for kt in range(NKT):
                            nc.tensor.matmul(
                                ctx[:],
                                V_sb[:, kt * VW + h * 65:
                                     kt * VW + (h + 1) * 65],
                                probs[:, kt * 2 * TOK + a * TOK:
                                      kt * 2 * TOK + (a + 1) * TOK],
                                start=(kt == 0), stop=(kt == NKT - 1))
                        # normalize: recip of sums (row 64) -> broadcast
                        s64 = sb.tile([65, TOK], F32, tag="s64", bufs=3,
                                      name=f"s64_{l}_{h}")
                        nc.vector.reciprocal(s64[64:65, :], ctx[64:65, :])
                        stile = sb.tile([1, TOK], F32, tag="stile", bufs=1,
                                        name=f"sums_{l}_{h}")
                        nc.sync.dma_start(r32(stile[:]), r32(s64[64:65, :]))
                        rb = psp.tile([64, TOK], F32, tag="ps1", bufs=2,
                                      name=f"rb_{l}_{h}")
                        nc.tensor.matmul(rb[:], r32(ones_row[:, 0:64]),
                                         r32(stile[:]))
                        rb_sb = sb.tile([64, TOK], F32, tag="s64", bufs=3,
                                        name=f"rbsb_{l}_{h}")
                        nc.vector.tensor_copy(rb_sb[:], rb[:])
                        if a == 0:
                            nc.vector.tensor_mul(st[0:64, :], ctx[0:64, :],
                                                 rb_sb[:])
                        else:
                            ch = sb.tile([64, TOK], BF16, tag="chodd",
                                         bufs=1, name=f"ch_{l}_{h}")
                            nc.vector.tensor_mul(ch[:], ctx[0:64, :],
                                                 rb_sb[:])
                            nc.sync.dma_start(st[64:128, :], ch[:])
                    stk.append(st)

                # ---- Wo projection (K=128 per pair) + residual -> r1 ----
                r1 = []
                for o in range(KD):
                    wot = sb.tile([P, 1024], BF16, tag="wproj", bufs=2,
                                  name=f"wo_{l}_{o}")
                    nc.sync.dma_start(wot[:], wo[l, o])
                    ps = psp.tile([P, TOK], F32, tag="ps1", bufs=2,
                                  name=f"ps_wo_{l}_{o}")
                    for p in range(NPAIR):
                        nc.tensor.matmul(
                            ps[:], wot[:, p * P:(p + 1) * P], stk[p][:],
                            start=(p == 0), stop=(p == NPAIR - 1))
                    tsy = sb.tile([P, TOK], F32, tag="tsy", bufs=3,
                                  name=f"tsy_{l}_{o}")
                    nc.vector.tensor_scalar(
                        tsy[:], xh[o][:], vec_t[:, C_G + o:C_G + o + 1],
                        vec_t[:, C_BY + o:C_BY + o + 1], ALU.mult, ALU.add)
                    rt = sb.tile([P, TOK], BF16, tag=f"r{o}", bufs=1,
                                 name=f"r1_{l}_{o}")
                    nc.vector.tensor_add(rt[:], ps[:], tsy[:])
                    r1.append(rt)

                def layernorm(src, phase):
                    """src: 8 bf16 tiles [128, TOK] -> (xh f32, xb bf16)."""
                    stats = psp.tile([1, 2 * TOK], F32, tag="lg", bufs=2,
                                     name=f"stats_{phase}")
                    for k in range(KD):
                        nc.tensor.matmul(stats[:, 0:TOK], ones_cb[:],
                                         src[k][:],
                                         start=(k == 0), stop=(k == KD - 1))
                    for k in range(KD):
                        sq = sb.tile([P, TOK], F32, tag="tsy", bufs=3,
                                     name=f"sq_{phase}_{k}")
                        nc.scalar.square(r32(sq[:]), src[k][:])
                        nc.tensor.matmul(stats[:, TOK:2 * TOK],
                                         r32(ones_col[:]), r32(sq[:]),
                                         start=(k == 0), stop=(k == KD - 1))
                    stA = sb.tile([1, 3 * TOK], F32, tag="st", bufs=1,
                                  name=f"stA_{phase}")
                    stB = sb.tile([1, 2 * TOK], F32, tag="stB", bufs=1,
                                  name=f"stB_{phase}")
                    mean = stA[:, 0:TOK]
                    wk1 = stA[:, TOK:2 * TOK]     # msq -> var -> ln(var)
                    t3 = stA[:, 2 * TOK:3 * TOK]
                    rs = stB[:, 0:TOK]
                    murs = stB[:, TOK:2 * TOK]
                    nc.vector.tensor_scalar_mul(mean, stats[:, 0:TOK], 1.0 / D)
                    nc.vector.tensor_scalar_mul(wk1, stats[:, TOK:2 * TOK],
                                                1.0 / D)
                    nc.vector.tensor_mul(t3, mean, mean)
                    nc.vector.tensor_sub(wk1, wk1, t3)           # var
                    nc.vector.tensor_scalar_add(wk1, wk1, EPS)
                    nc.scalar.activation(wk1, wk1, AF.Ln)
                    nc.scalar.activation(r32(rs), wk1, AF.Exp, scale=-0.5)
                    nc.vector.tensor_mul(r32(murs), mean, rs)    # mu*rs
                    bc = psp.tile([P, 2 * TOK], F32, tag="lg", bufs=2,
                                  name=f"bc_{phase}")
                    nc.tensor.matmul(bc[:, 0:TOK], r32(ones_row[:]), r32(rs))
                    nc.tensor.matmul(bc[:, TOK:2 * TOK], r32(ones_row[:]),
                                     r32(murs))
                    outs = []
                    outsb = []
                    for k in range(KD):
                        tb = sb.tile([P, TOK], F32, tag="tsy", bufs=3,
                                     name=f"lntmp_{phase}_{k}")
                        nc.vector.tensor_mul(tb[:], src[k][:], bc[:, 0:TOK])
                        nt = sb.tile([P, TOK], F32, tag=f"x{k}", bufs=1,
                                     name=f"xh_{phase}_{k}")
                        nc.vector.tensor_sub(r32(nt[:]), tb[:],
                                             bc[:, TOK:2 * TOK])
                        outs.append(nt)
                        nb = sb.tile([P, TOK], BF16, tag=f"xb{k}", bufs=1,
                                     name=f"xb_{phase}_{k}")
                        nc.vector.tensor_copy(nb[:], nt[:])
                        outsb.append(nb)
                    return outs, outsb

                xh, xb = layernorm(r1, f"ln1_{l}")

                # ---- FFN ----
                h_sb = []
                for f in range(KF):
                    wt = sb.tile([P, D], BF16, tag="wfc1", bufs=2,
                                 name=f"w1_{l}_{f}")
                    nc.sync.dma_start(wt[:], w1[l, f])
                    ps = psp.tile([P, TOK], F32, tag="ps1", bufs=2,
                                  name=f"ps_f1_{l}_{f}")
                    for k in range(KD):
                        nc.tensor.matmul(ps[:], wt[:, k * P:(k + 1) * P],
                                         xb[k][:],
                                         start=(k == 0), stop=(k == KD - 1))
                    ht = sb.tile([P, TOK], BF16, tag="hsb", bufs=32,
                                 name=f"h_{l}_{f}")
                    nc.scalar.activation(
                        ht[:], ps[:], AF.Gelu,
                        bias=vec_t[:, C_F1B + f:C_F1B + f + 1])
                    h_sb.append(ht)
                r2 = []
                for o in range(KD):
                    ps = psp.tile([P, TOK], F32, tag="ps1", bufs=2,
                                  name=f"ps_f2_{l}_{o}")
                    for qt in range(4):
                        wt = sb.tile([P, 1024], BF16, tag="wfc2", bufs=2,
                                     name=f"w2_{l}_{o}_{qt}")
                        nc.sync.dma_start(wt[:], w2[l, o, qt])
                        for k in range(8):
                            kk = qt * 8 + k
                            nc.tensor.matmul(ps[:],
                                             wt[:, k * P:(k + 1) * P],
                                             h_sb[kk][:],
                                             start=(kk == 0), stop=(kk == 31))
                    tsy = sb.tile([P, TOK], F32, tag="tsy", bufs=3,
                                  name=f"tsy1_{l}_{o}")
                    nc.vector.tensor_scalar(
                        tsy[:], xh[o][:], vec_t[:, C_G1 + o:C_G1 + o + 1],
                        vec_t[:, C_BY1 + o:C_BY1 + o + 1], ALU.mult, ALU.add)
                    rt = sb.tile([P, TOK], BF16, tag=f"r{o}", bufs=1,
                                 name=f"r2_{l}_{o}")
                    nc.vector.tensor_add(rt[:], ps[:], tsy[:])
                    r2.append(rt)

                xh, xb = layernorm(r2, f"ln2_{l}")

            # ---- final: y3 = Gf*xh + BYf ; LNf ; apply lnf_g/b ; store ----
            vecf_t = sb.tile([P, 32], F32, tag="vec", bufs=2)
            nc.sync.dma_start(vecf_t[:], vecf[:])
            y3 = []
            for k in range(KD):
                yt = sb.tile([P, TOK], BF16, tag=f"r{k}", bufs=1,
                             name=f"y3_{k}")
                nc.vector.tensor_scalar(
                    yt[:], xh[k][:], vecf_t[:, k:k + 1],
                    vecf_t[:, 8 + k:8 + k + 1], ALU.mult, ALU.add)
                y3.append(yt)

            stats = psp.tile([1, 2 * TOK], F32, tag="lg", bufs=2,
                             name="stats_f")
            for k in range(KD):
                nc.tensor.matmul(stats[:, 0:TOK], ones_cb[:], y3[k][:],
                                 start=(k == 0), stop=(k == KD - 1))
            for k in range(KD):
                sq = sb.tile([P, TOK], F32, tag="tsy", bufs=3,
                             name=f"sq_f_{k}")
                nc.scalar.square(r32(sq[:]), y3[k][:])
                nc.tensor.matmul(stats[:, TOK:2 * TOK], r32(ones_col[:]),
                                 r32(sq[:]),
                                 start=(k == 0), stop=(k == KD - 1))
            stA = sb.tile([1, 3 * TOK], F32, tag="st", bufs=1, name="stA_f")
            stB = sb.tile([1, 2 * TOK], F32, tag="stB", bufs=1, name="stB_f")
            mean = stA[:, 0:TOK]
            wk1 = stA[:, TOK:2 * TOK]
            t3 = stA[:, 2 * TOK:3 * TOK]
            rs = stB[:, 0:TOK]
            murs = stB[:, TOK:2 * TOK]
            nc.vector.tensor_scalar_mul(mean, stats[:, 0:TOK], 1.0 / D)
            nc.vector.tensor_scalar_mul(wk1, stats[:, TOK:2 * TOK], 1.0 / D)
            nc.vector.tensor_mul(t3, mean, mean)
            nc.vector.tensor_sub(wk1, wk1, t3)
            nc.vector.tensor_scalar_add(wk1, wk1, EPS)
            nc.scalar.activation(wk1, wk1, AF.Ln)
            nc.scalar.activation(r32(rs), wk1, AF.Exp, scale=-0.5)
            nc.vector.tensor_mul(r32(murs), mean, rs)
            bc = psp.tile([P, 2 * TOK], F32, tag="lg", bufs=2, name="bc_f")
            nc.tensor.matmul(bc[:, 0:TOK], r32(ones_row[:]), r32(rs))
            nc.tensor.matmul(bc[:, TOK:2 * TOK], r32(ones_row[:]), r32(murs))
            for k in range(KD):
                tb = sb.tile([P, TOK], F32, tag="tsy", bufs=3,
                             name=f"ftmp_{k}")
                nc.vector.tensor_mul(tb[:], y3[k][:], bc[:, 0:TOK])
                ot = sb.tile([P, TOK], F32, tag="s64", bufs=3,
                             name=f"out_{k}")
                nc.vector.tensor_sub(ot[:], tb[:], bc[:, TOK:2 * TOK])
                ot2 = sb.tile([P, TOK], F32, tag="tsy", bufs=3,
                              name=f"out2_{k}")
                nc.vector.tensor_scalar(
                    ot2[:], ot[:], vecf_t[:, 16 + k:16 + k + 1],
                    vecf_t[:, 24 + k:24 + k + 1], ALU.mult, ALU.add)
                nc.sync.dma_start(outT[k], ot2[:])

    nc.compile()
    return nc


def _get_program():
    global _PROGRAM
    if _PROGRAM is None:
        _PROGRAM = _build_program()
    return _PROGRAM


def _prep_host(inputs):
    """Fold affines/biases into weights; build DMA-friendly bf16 layouts."""
    f = lambda a: np.asarray(a, dtype=np.float64)
    Wq, bq = f(inputs["Wq"]), f(inputs["bq"])
    Wk, bk = f(inputs["Wk"]), f(inputs["bk"])
    Wv, bv = f(inputs["Wv"]), f(inputs["bv"])
    Wo, bo = f(inputs["Wo"]), f(inputs["bo"])
    f1w, f1b = f(inputs["fc1_w"]), f(inputs["fc1_b"])
    f2w, f2b = f(inputs["fc2_w"]), f(inputs["fc2_b"])
    ln1_g, ln1_b = f(inputs["ln1_g"]), f(inputs["ln1_b"])
    ln2_g, ln2_b = f(inputs["ln2_g"]), f(inputs["ln2_b"])
    lnf_g, lnf_b = f(inputs["lnf_g"]), f(inputs["lnf_b"])
    BF = ml_dtypes.bfloat16

    def proj_tiles(WT):  # WT [D_in, M_out] -> [M_out/P][P, D_in] lhsT blocks
        kd = WT.shape[0] // P
        n_o = WT.shape[1] // P
        a = WT.reshape(kd, P, n_o, P).transpose(2, 1, 0, 3)  # [o, p, k, c]
        return np.ascontiguousarray(a.reshape(n_o, P, kd * P))

    wq_h = np.empty((L, KD, P, D), BF)
    wk_h = np.empty((L, KD, P, D), BF)
    wv_h = np.empty((L, KD, P, D), BF)
    wo_h = np.empty((L, KD, P, D), BF)
    w1_h = np.empty((L, KF, P, D), BF)
    w2_h = np.empty((L, KD, 4, P, 1024), BF)
    vecs_h = np.empty((L, P, NVEC), np.float32)

    def cols(v):  # [dim] -> [P, dim/P] per-d-tile columns
        return np.ascontiguousarray(v.reshape(-1, P).T.astype(np.float32))

    g_in = np.ones(D)
    b_in = np.zeros(D)
    for l in range(L):
        bvp = bv[l] + Wv[l] @ b_in
        b_att = bo[l] + Wo[l] @ bvp
        bqp = bq[l] + Wq[l] @ b_in
        bkp = bk[l] + Wk[l] @ b_in
        f1bp = f1b[l] + f1w[l] @ ln1_b[l]
        wq_h[l] = proj_tiles((Wq[l] * g_in[None, :]).T)
        wk_h[l] = proj_tiles((Wk[l] * g_in[None, :]).T)
        # wv used as rhs tiles: wv_h[l, k] = WvT'[kP:(k+1)P, :]
        wv_h[l] = (Wv[l] * g_in[None, :]).T.reshape(KD, P, D)
        # wo lhsT blocks: wo_h[l, o][r, p*128 + c] = WoT[p*128 + r, o*128 + c]
        WoT = Wo[l].T  # [1024 ctx-dims, 1024 out]
        wo_h[l] = (WoT.reshape(8, 128, KD, P)      # [p, r, o, c]
                   .transpose(2, 1, 0, 3).reshape(KD, P, D))
        w1_h[l] = proj_tiles((f1w[l] * ln1_g[l][None, :]).T)
        t2 = proj_tiles(f2w[l].T)  # [KD out][P, DFF]
        w2_h[l] = t2.reshape(KD, P, 4, 1024).transpose(0, 2, 1, 3)
        v = vecs_h[l]
        v[:, C_BQ:C_BQ + 8] = cols(bqp)
        v[:, C_BK:C_BK + 8] = cols(bkp)
        v[:, C_G:C_G + 8] = cols(g_in)
        v[:, C_BY:C_BY + 8] = cols(b_in + b_att)
        v[:, C_F1B:C_F1B + 32] = cols(f1bp)
        v[:, C_G1:C_G1 + 8] = cols(ln1_g[l])
        v[:, C_BY1:C_BY1 + 8] = cols(ln1_b[l] + f2b[l])
        g_in, b_in = ln2_g[l], ln2_b[l]

    vecf_h = np.empty((P, 32), np.float32)
    vecf_h[:, 0:8] = cols(g_in)
    vecf_h[:, 8:16] = cols(b_in)
    vecf_h[:, 16:24] = cols(lnf_g)
    vecf_h[:, 24:32] = cols(lnf_b)

    return dict(wq=wq_h, wk=wk_h, wv=wv_h, wo=wo_h, w1=w1_h, w2=w2_h,
                vecs=vecs_h, vecf=vecf_h)


def kernel(**inputs):
    nc = _get_program()
    shared = _prep_host(inputs)
    x = np.asarray(inputs["x"], dtype=np.float32)
    mask = np.asarray(inputs["mask"])

    in_maps = []
    for c in range(NCORES):
        b, s = c // GROUP, c % GROUP
        xTc = np.ascontiguousarray(
            x[b, s * TOK:(s + 1) * TOK, :].T).reshape(KD, P, TOK)
        # -2.0: shift exp into fp8e4m3 range (cancels in the normalization)
        mbv = (mask[b].astype(np.float64) - 1.0) * 30.0 - 2.0
        mb_c = np.ascontiguousarray(mbv.reshape(NKT, P).T.astype(np.float32))
        m = dict(shared)
        m["xT"] = xTc
        m["mb"] = mb_c
        m["onesd"] = np.ones((P, 8), np.float32)
        in_maps.append(m)

    global LAST
    res = run_bass_kernel_spmd(nc, in_maps, list(range(NCORES)))
    LAST = res
    out = np.empty((B, T, D), np.float32)
    for c in range(NCORES):
        b, s = c // GROUP, c % GROUP
        oT = res.results[c]["outT"].reshape(D, TOK)
        out[b, s * TOK:(s + 1) * TOK, :] = oT.T
    return out
